# revision 8
# baseline (speedup 1.0000x reference)
"""Trainium2 Bass kernel for nn_Attention_88510686036327.

GQA attention (S=2048, DIM=4096, H=32 q-heads, KVH=8 kv-heads, D=128) with
RoPE and causal softmax, tensor-parallel across 8 NeuronCores: each core owns
1 kv-head + 4 q-heads (wq/wk/wv sharded on head dim, wo on input dim), and the
partial wo outputs are summed on the host.

Numerics (reference multiplies scores by sqrt(D), making logits ~N(0, 345) —
softmax is near-argmax, so the q/k path needs fp32-grade precision):
  - Q/K projections and QK^T: fp16 hi/lo split, 3 matmuls (error ~1e-4 coherent)
  - V projection, P (probs), PV, wo: plain fp16 (error ~2.4e-4, incoherent)
  - RoPE, softmax: fp32 on DVE/ACT engines
  - weights pre-scaled by 64 on host so fp16 lo-parts stay out of subnormals;
    descale folded into existing eviction ops
  - softmax normalization folded into the P-transpose matmul by replacing the
    transpose identity with diag(1/rowsum)

Layouts (host-prepped): x, wq, wk, wv transposed so the contraction dim lands
on SBUF partitions with no on-device transposes; q/k/v produced directly in
the layouts the attention matmuls need.
"""
import sys

sys.path.insert(0, "/opt/trn_rl_repo")

import numpy as np

S = 2048
DIM = 4096
H = 32
KVH = 8
D = 128
N_CORES = 8
HPC = H // N_CORES          # q heads per core
MQ = HPC * D                # per-core q rows (512)
KT = DIM // 128             # contraction tiles (32)
SC = S // 512               # s-chunks (4)
WSCALE = 64.0
SQRT_D = float(np.sqrt(D))
NEG = -1e30

_CACHE = {}
LAST_RESULT = None


def _build():
    import concourse.bacc as bacc
    import concourse.mybir as mybir
    import concourse.tile as tile

    dt = mybir.dt
    f16, f32 = dt.float16, dt.float32
    AX = mybir.AxisListType.X
    SUB = mybir.AluOpType.subtract
    ADD = mybir.AluOpType.add
    EXP = mybir.ActivationFunctionType.Exp
    LN = mybir.ActivationFunctionType.Ln

    nc = bacc.Bacc("TRN2", target_bir_lowering=False, debug=False)

    def din(name, shape, d=f16):
        return nc.dram_tensor(name, shape, d, kind="ExternalInput").ap()

    xh_d = din("xh", [DIM, S])
    xl_d = din("xl", [DIM, S])
    wqh_d = din("wqh", [DIM, MQ])
    wql_d = din("wql", [DIM, MQ])
    wkh_d = din("wkh", [DIM, D])
    wkl_d = din("wkl", [DIM, D])
    wv_d = din("wv", [DIM, D])
    wo_d = din("wot", [MQ, DIM])
    cos_d = din("cosf", [D, S], f32)
    sin_d = din("sinf", [D, S], f32)
    mask_d = din("masks", [4, 128, 512], f32)
    id_d = din("ident", [128, 128], f16)
    rm_d = din("rmat", [128, 128], f32)
    y_d = nc.dram_tensor("y", [S, DIM], f16, kind="ExternalOutput").ap()

    xh_r = xh_d.rearrange("(kt p) s -> p kt s", p=128)
    xl_r = xl_d.rearrange("(kt p) s -> p kt s", p=128)
    wqh_r = wqh_d.rearrange("(kt p) m -> p kt m", p=128)
    wql_r = wql_d.rearrange("(kt p) m -> p kt m", p=128)
    wkh_r = wkh_d.rearrange("(kt p) m -> p kt m", p=128)
    wkl_r = wkl_d.rearrange("(kt p) m -> p kt m", p=128)
    wv_r = wv_d.rearrange("(kt p) m -> p kt m", p=128)
    wo_r = wo_d.rearrange("(kd p) n -> p kd n", p=128)

    with tile.TileContext(nc) as tc:
        with tc.tile_pool(name="persist", bufs=1) as pp:
            ident = pp.tile([128, 128], f16, name="ident")
            nc.sync.dma_start(ident[:], id_d)
            rmat = pp.tile([128, 128], f32, name="rmat")
            nc.sync.dma_start(rmat[:], rm_d)
            maskt = []
            for j in range(4):
                mj = pp.tile([128, 512], f32, name=f"mask{j}", tag=f"mask{j}")
                maskt.append(mj)
            qh_s = pp.tile([128, HPC, S], f16, name="qh_s")
            ql_s = pp.tile([128, HPC, S], f16, name="ql_s")
            kh_s = pp.tile([128, S], f16, name="kh_s")
            kl_s = pp.tile([128, S], f16, name="kl_s")
            v_s = pp.tile([128, 16, 128], f16, name="v_s")

            # ---------------- phase 1: projections + rope ----------------
            with (
                tc.tile_pool(name="p1w", bufs=1) as p1w,
                tc.tile_pool(name="p1x", bufs=3) as p1x,
                tc.tile_pool(name="p1r", bufs=2) as p1r,
                tc.tile_pool(name="ps1", bufs=1, space="PSUM") as ps1,
            ):
                wqh = p1w.tile([128, KT, MQ], f16, name="wqh")
                wql = p1w.tile([128, KT, MQ], f16, name="wql")
                wkh = p1w.tile([128, KT, D], f16, name="wkh")
                wkl = p1w.tile([128, KT, D], f16, name="wkl")
                wv = p1w.tile([128, KT, D], f16, name="wv")
                cosf = p1w.tile([128, S], f32, name="cosf")
                sinf = p1w.tile([128, S], f32, name="sinf")

                def rope_unit(psum, outh, outl, ss):
                    """psum [128,512] raw proj -> rope'd hi/lo fp16 slices."""
                    sb = p1r.tile([128, 512], f32, name="ropesb", tag="ropesb")
                    nc.scalar.mul(sb[:], psum[:], 1.0 / WSCALE)
                    sw = ps1.tile([128, 512], f32, name="ropesw", tag="ropesw")
                    nc.tensor.matmul(sw[:], rmat[:], sb[:], start=True, stop=True)
                    t1 = p1r.tile([128, 512], f32, name="ropet1", tag="ropet1")
                    nc.vector.tensor_mul(t1[:], sb[:], cosf[:, ss])
                    t2 = p1r.tile([128, 512], f32, name="ropet2", tag="ropet2")
                    nc.vector.tensor_mul(t2[:], sw[:], sinf[:, ss])
                    nc.vector.tensor_add(outh, t1[:], t2[:])  # fp16 hi
                    t3 = p1r.tile([128, 512], f32, name="ropet3", tag="ropet3")
                    nc.vector.tensor_tensor(t3[:], t2[:], outh, SUB)
                    nc.vector.tensor_tensor(outl, t1[:], t3[:], ADD)  # fp16 lo

                for sc in range(SC):
                    ss = slice(sc * 512, (sc + 1) * 512)
                    qps = [
                        ps1.tile([128, 512], f32, name=f"qps{m}", tag=f"qps{m}")
                        for m in range(HPC)
                    ]
                    kps = ps1.tile([128, 512], f32, name="kps", tag="kps")
                    vps = ps1.tile([128, 512], f32, name="vps", tag="vps")
                    for kt in range(KT):
                        first, last = kt == 0, kt == KT - 1
                        if sc == 0:
                            nc.sync.dma_start(wqh[:, kt, :], wqh_r[:, kt, :])
                            nc.sync.dma_start(wql[:, kt, :], wql_r[:, kt, :])
                            nc.sync.dma_start(wkh[:, kt, :], wkh_r[:, kt, :])
                            nc.sync.dma_start(wkl[:, kt, :], wkl_r[:, kt, :])
                            nc.sync.dma_start(wv[:, kt, :], wv_r[:, kt, :])
                            if kt == 8:
                                nc.sync.dma_start(cosf[:], cos_d)
                                nc.sync.dma_start(sinf[:], sin_d)
                        xht = p1x.tile([128, 512], f16, name="xht", tag="xht")
                        nc.sync.dma_start(xht[:], xh_r[:, kt, ss])
                        xlt = p1x.tile([128, 512], f16, name="xlt", tag="xlt")
                        nc.sync.dma_start(xlt[:], xl_r[:, kt, ss])
                        for m in range(HPC):
                            wh = wqh[:, kt, m * 128 : (m + 1) * 128]
                            wl = wql[:, kt, m * 128 : (m + 1) * 128]
                            nc.tensor.matmul(qps[m][:], wh, xht[:], start=first, stop=False)
                            nc.tensor.matmul(qps[m][:], wh, xlt[:], start=False, stop=False)
                            nc.tensor.matmul(qps[m][:], wl, xht[:], start=False, stop=last)
                        nc.tensor.matmul(kps[:], wkh[:, kt, :], xht[:], start=first, stop=False)
                        nc.tensor.matmul(kps[:], wkh[:, kt, :], xlt[:], start=False, stop=False)
                        nc.tensor.matmul(kps[:], wkl[:, kt, :], xht[:], start=False, stop=last)
                        nc.tensor.matmul(vps[:], wv[:, kt, :], xht[:], start=first, stop=last)
                    for m in range(HPC):
                        rope_unit(qps[m], qh_s[:, m, ss], ql_s[:, m, ss], ss)
                    rope_unit(kps, kh_s[:, ss], kl_s[:, ss], ss)
                    # V: evict fp16 then transpose to natural [s, d] layout
                    vsb = p1r.tile([128, 512], f16, name="vsb", tag="vsb")
                    nc.scalar.mul(vsb[:], vps[:], 1.0 / WSCALE)
                    vtp = ps1.tile([128, 512], f16, name="vtp", tag="vtp")
                    for j in range(4):
                        nc.tensor.transpose(
                            vtp[:, j * 128 : (j + 1) * 128],
                            vsb[:, j * 128 : (j + 1) * 128],
                            ident[:],
                        )
                    nc.vector.tensor_copy(v_s[:, sc * 4 : (sc + 1) * 4, :], vtp[:])

            # ---------------- phase 2: attention + wo ----------------
            with (
                tc.tile_pool(name="p2w", bufs=1) as p2w,
                tc.tile_pool(name="p2a", bufs=2) as p2a,
                tc.tile_pool(name="p2x", bufs=3) as p2x,
                tc.tile_pool(name="ps2", bufs=2, space="PSUM") as ps2,
            ):
                for j in range(4):
                    nc.sync.dma_start(maskt[j][:], mask_d[j])
                wot = p2w.tile([128, HPC, DIM], f16, name="wot")
                nc.sync.dma_start(wot[:], wo_r)
                for qc in range(SC):
                    attn_t = []
                    for h in range(HPC):
                        nkc = qc + 1
                        exps = []
                        for qtl in range(4):
                            qt = qc * 4 + qtl
                            qsl = slice(qt * 128, (qt + 1) * 128)
                            scsb = p2a.tile([128, S], f32, name="scsb", tag="scsb")
                            mbuf = p2a.tile([128, 4], f32, name="mbuf", tag="mbuf")
                            qh_sl = qh_s[:, h, qsl]
                            ql_sl = ql_s[:, h, qsl]
                            for kc in range(nkc):
                                ks = slice(kc * 512, (kc + 1) * 512)
                                sps = ps2.tile([128, 512], f32, name="sps", tag="sps")
                                nc.tensor.matmul(sps[:], qh_sl, kh_s[:, ks], start=True, stop=False)
                                nc.tensor.matmul(sps[:], qh_sl, kl_s[:, ks], start=False, stop=False)
                                nc.tensor.matmul(sps[:], ql_sl, kh_s[:, ks], start=False, stop=True)
                                if kc == qc:
                                    nc.vector.tensor_tensor(scsb[:, ks], sps[:], maskt[qtl][:], ADD)
                                else:
                                    nc.scalar.copy(scsb[:, ks], sps[:])
                                nc.vector.reduce_max(mbuf[:, kc : kc + 1], scsb[:, ks], axis=AX)
                            mrow = p2a.tile([128, 1], f32, name="mrow", tag="mrow")
                            nc.vector.reduce_max(mrow[:], mbuf[:, 0:nkc], axis=AX)
                            bias = p2a.tile([128, 1], f32, name="bias", tag="bias")
                            nc.vector.tensor_scalar_mul(bias[:], mrow[:], -SQRT_D)
                            expt = p2a.tile([128, S], f16, name=f"exp{qtl}", tag=f"exp{qtl}")
                            sebuf = p2a.tile([128, 4], f32, name="sebuf", tag="sebuf")
                            for kc in range(nkc):
                                ks = slice(kc * 512, (kc + 1) * 512)
                                nc.scalar.activation(
                                    expt[:, ks],
                                    scsb[:, ks],
                                    EXP,
                                    bias=bias[:],
                                    scale=SQRT_D,
                                    accum_out=sebuf[:, kc : kc + 1],
                                )
                            tot = p2a.tile([128, 1], f32, name="tot", tag="tot")
                            nc.vector.reduce_sum(tot[:], sebuf[:, 0:nkc], axis=AX)
                            rr = p2a.tile([128, 1], f32, name="rr", tag="rr")
                            nc.vector.reciprocal(rr[:], tot[:])
                            nc.vector.tensor_scalar_mul(
                                expt[:, 0 : nkc * 512], expt[:, 0 : nkc * 512], rr[:]
                            )
                            exps.append(expt)
                        # P transpose (+normalize via diag) and PV, 1-step pipelined
                        ops_ = ps2.tile([128, 512], f32, name="ops", tag="ops")
                        nkt = 4 * (qc + 1)
                        pts = [None] * nkt
                        for kt in range(nkt):
                            tps = ps2.tile([128, 512], f16, name="tps", tag="tps")
                            for qtl in range(4):
                                nc.tensor.transpose(
                                    tps[:, qtl * 128 : (qtl + 1) * 128],
                                    exps[qtl][:, kt * 128 : (kt + 1) * 128],
                                    ident[:],
                                )
                            pt = p2x.tile([128, 512], f16, name="pt", tag="pt")
                            nc.vector.tensor_copy(pt[:], tps[:])
                            pts[kt] = pt
                            if kt > 0:
                                nc.tensor.matmul(
                                    ops_[:], v_s[:, kt - 1, :], pts[kt - 1][:],
                                    start=(kt == 1), stop=False,
                                )
                        nc.tensor.matmul(
                            ops_[:], v_s[:, nkt - 1, :], pts[nkt - 1][:],
                            start=(nkt == 1), stop=True,
                        )
                        at = p2a.tile([128, 512], f16, name=f"attn{h}", tag=f"attn{h}")
                        nc.vector.tensor_copy(at[:], ops_[:])
                        attn_t.append(at)
                    # wo for this q-chunk
                    for stl in range(4):
                        sl = slice(stl * 128, (stl + 1) * 128)
                        row0 = qc * 512 + stl * 128
                        for dc in range(8):
                            yps = ps2.tile([128, 512], f32, name="yps", tag="yps")
                            for h in range(HPC):
                                nc.tensor.matmul(
                                    yps[:],
                                    attn_t[h][:, sl],
                                    wot[:, h, dc * 512 : (dc + 1) * 512],
                                    start=(h == 0),
                                    stop=(h == HPC - 1),
                                )
                            ysb = p2x.tile([128, 512], f16, name="ysb", tag="ysb")
                            nc.scalar.mul(ysb[:], yps[:], 1.0 / WSCALE)
                            nc.sync.dma_start(
                                y_d[row0 : row0 + 128, dc * 512 : (dc + 1) * 512], ysb[:]
                            )
    nc.compile()
    return nc


def _get_nc():
    if "nc" not in _CACHE:
        _CACHE["nc"] = _build()
    return _CACHE["nc"]


def _prep_inputs(x, wq, wk, wv, wo, freqs_cis):
    """Host-side shard + layout prep. Returns in_maps (one dict per core)."""
    f16 = np.float16
    f32 = np.float32
    x = np.asarray(x, f32)
    wq = np.asarray(wq, f32)
    wk = np.asarray(wk, f32)
    wv = np.asarray(wv, f32)
    wo = np.asarray(wo, f32)
    fc = np.asarray(freqs_cis, f32)

    xT = np.ascontiguousarray(x.T)                      # [DIM, S]
    xh = xT.astype(f16)
    xl = (xT - xh.astype(f32)).astype(f16)

    # rope tables in [d, s] layout; sin carries the pair-swap signs
    cosf = np.empty((D, S), f32)
    sinf = np.empty((D, S), f32)
    c = fc[:, :, 0].T                                   # [64, S]
    s = fc[:, :, 1].T
    cosf[0::2] = c
    cosf[1::2] = c
    sinf[0::2] = -s
    sinf[1::2] = s

    masks = np.empty((4, 128, 512), f32)
    q_i = np.arange(128)[:, None]
    k_i = np.arange(512)[None, :]
    for j in range(4):
        masks[j] = np.where(k_i <= 128 * j + q_i, 0.0, NEG)

    ident = np.eye(128, dtype=f16)
    rmat = np.zeros((128, 128), f32)
    ii = np.arange(0, 128, 2)
    rmat[ii + 1, ii] = 1.0   # lhsT[2i+1, 2i]=1 -> out[2i] = in[2i+1]
    rmat[ii, ii + 1] = 1.0   # lhsT[2i, 2i+1]=1 -> out[2i+1] = in[2i]

    in_maps = []
    for cidx in range(N_CORES):
        hs = slice(cidx * MQ, (cidx + 1) * MQ)
        ks = slice(cidx * D, (cidx + 1) * D)
        wqT = np.ascontiguousarray((wq[hs] * WSCALE).T)  # [DIM, 512]
        wqh = wqT.astype(f16)
        wql = (wqT - wqh.astype(f32)).astype(f16)
        wkT = np.ascontiguousarray((wk[ks] * WSCALE).T)  # [DIM, 128]
        wkh = wkT.astype(f16)
        wkl = (wkT - wkh.astype(f32)).astype(f16)
        wvT = np.ascontiguousarray((wv[ks] * WSCALE).T).astype(f16)
        woT = np.ascontiguousarray((wo[:, hs] * WSCALE).T).astype(f16)  # [512, DIM]
        in_maps.append(
            {
                "xh": xh, "xl": xl,
                "wqh": wqh, "wql": wql,
                "wkh": wkh, "wkl": wkl,
                "wv": wvT, "wot": woT,
                "cosf": cosf, "sinf": sinf,
                "masks": masks, "ident": ident, "rmat": rmat,
            }
        )
    return in_maps


def kernel(**inputs):
    global LAST_RESULT
    from concourse.bass_utils import run_bass_kernel_spmd

    in_maps = _prep_inputs(
        inputs["x"], inputs["wq"], inputs["wk"], inputs["wv"], inputs["wo"],
        inputs["freqs_cis"],
    )
    nc = _get_nc()
    r = run_bass_kernel_spmd(nc, in_maps, core_ids=list(range(N_CORES)))
    LAST_RESULT = r
    y = np.zeros((S, DIM), np.float32)
    for cidx in range(N_CORES):
        y += r.results[cidx]["y"].astype(np.float32)
    return y


if __name__ == "__main__":
    rng = np.random.default_rng(0)
    demo = {
        "x": rng.standard_normal((S, DIM)).astype(np.float32),
        "wq": (rng.standard_normal((H * D, DIM)) * 0.02).astype(np.float32),
        "wk": (rng.standard_normal((KVH * D, DIM)) * 0.02).astype(np.float32),
        "wv": (rng.standard_normal((KVH * D, DIM)) * 0.02).astype(np.float32),
        "wo": (rng.standard_normal((DIM, H * D)) * 0.02).astype(np.float32),
        "freqs_cis": np.stack(
            [
                np.cos(np.outer(np.arange(S), 1.0 / 10000.0 ** (np.arange(0, D, 2) / D))),
                np.sin(np.outer(np.arange(S), 1.0 / 10000.0 ** (np.arange(0, D, 2) / D))),
            ],
            axis=-1,
        ).astype(np.float32),
    }
    y = kernel(**demo)
    print("ok", y.shape, y.dtype)
